# revision 73
# baseline (speedup 1.0000x reference)
"""Multi-head causal self-attention on 8 Trainium2 NeuronCores.

Problem: x[4,2048,1024] @ w_qkv[1024,3072] -> 16-head causal attention
         -> @ w_out[1024,1024] + b_out.

Sharding (hardcoded): 8 cores = 4 batches x 2 head-groups of 8 heads.
Core c handles batch b = c//2 and heads hg*8..hg*8+8, hg = c%2.
Each core computes a partial output [2048,1024] (its 8 heads pushed
through its w_out row-slice); host sums the two head-group partials per
batch and adds b_out.

Everything computes in fp16 (10 mantissa bits; fp32 PSUM accumulation),
which runs matmuls at full 1 cycle/row PE rate and lands ~7e-4 relative
error vs the fp32 reference.

Device algorithm per core (all "transposed orientation" so the only
transpose needed -- x^T -- is done for free on the host):
  qT/kT [512, 2048] and v (natural [2048, 512]) via fp16 matmuls.
  Per pair GROUP (2 head pairs), per 512-wide query chunk: the two
  pairs' QK/exp streams interleave per 128-key tile:
    scores^T[j,i] for both heads of a pair into one 2-bank PSUM tile
    via row-tiled (K=64) matmul pairs, fully narrowed to the causal
    range; ONE exp per (pair, key-tile) on ScalarE, with a 3D AP that
    skips the dead zone between the two heads' halves on diagonal
    tiles; causal masking via one 3D-AP mask multiply covering both
    heads' 128-wide triangle bands; softmax partials accumulated into
    one [128,1024] pacc tile per pair with one 3D-AP add.
    Pair A's PV (out^T[d,i] += col-tiled matmuls, PSUM accum over j)
    runs inline but lagged TWO key tiles behind its exp, so the
    in-order PE queue never stalls mid-step on an exp semaphore (the
    one pacing wait consolidates at the scores-PSUM ring alloc); pair
    B's pT tiles stay in SBUF and its PV runs as one dense burst
    after the loop.  Together these halve the number of
    dependency-exposed PE slots.  Denominators: 2 ones-matmuls per
    pair reduce pacc partitions into a shared den bank.
  1/denom via one reciprocal_approx_fast per group (reads the den
  PSUM bank directly), cast to fp16, then partition-broadcast with a
  single selection-matrix matmul per pair (sel.T @ recip rows) written
  back over the den bank; in-place normalize multiply reads PSUM.
  partial = att^T.T @ w_out_slice -> DMA to DRAM.

Scheduling: one global software pipeline.  phase_b(s) (attention for
query chunk s) is emitted step-by-step with QKV chains for stage s+1
and out-projection chains for earlier stages interleaved as PE filler,
so the Tensor engine stays dense while ScalarE churns through exps
(keeps the HAM clock-gate at full rate).  Stage 0 is special-cased:
the first exps depend only on the pair-0 q/k chains, which consume
the head-critical DMAs (w_qk chunks + stage-0 x^T slices, ordered
first) incrementally, so exps start ~25us in instead of ~40us.
DMA-instruction issue costs ~600ns each on the SP engine, so the
input loads use the largest lines the layouts allow (2KB+).
"""

import os
import sys

import numpy as np

if "/opt/trn_rl_repo" not in sys.path:
    sys.path.insert(0, "/opt/trn_rl_repo")

B, T, C = 4, 2048, 1024
H, D = 16, 64
NCORES = 8
HPC = 8  # heads per core
PAIRS = 4  # head pairs per core
CCH = 8  # contraction chunks over C (1024/128)
ICH = 4  # i (query) chunks of 512
NJT = 16  # j (key) tiles of 128

_CACHE = {}


def _build_program():
    import concourse.mybir as mybir
    import concourse.tile as tile
    from concourse import bacc

    f32 = mybir.dt.float32
    f16 = mybir.dt.float16
    EXP = mybir.ActivationFunctionType.Exp

    nc = bacc.Bacc(
        "TRN2", target_bir_lowering=False, debug=False, num_devices=NCORES
    )
    xt = nc.dram_tensor("xt", [C, T], f16, kind="ExternalInput").ap()
    wqk = nc.dram_tensor("wqk", [C, 1024], f16, kind="ExternalInput").ap()
    wv = nc.dram_tensor("wv", [C, 512], f16, kind="ExternalInput").ap()
    wo = nc.dram_tensor("wo", [512, C], f16, kind="ExternalInput").ap()
    msk = nc.dram_tensor("msk", [128, 896], f16, kind="ExternalInput").ap()
    out = nc.dram_tensor("out", [T, C], f32, kind="ExternalOutput").ap()

    with tile.TileContext(nc) as tc:
        with (
            tc.tile_pool(name="wpool", bufs=1) as wpool,
            tc.tile_pool(name="wopool", bufs=1) as wopool,
            tc.tile_pool(name="xpool", bufs=1) as xpool,
            tc.tile_pool(name="qkpool", bufs=8) as qkpool,
            tc.tile_pool(name="vpool", bufs=16) as vpool,
            tc.tile_pool(name="apool", bufs=4) as apool,
            tc.tile_pool(name="ppool", bufs=12) as ppool,
            tc.tile_pool(name="cpool", bufs=1) as cpool,
            tc.tile_pool(name="rpool", bufs=4) as rpool,
            tc.tile_pool(name="qpool", bufs=4) as qpool,
            tc.tile_pool(name="opool", bufs=4) as opool,
            tc.tile_pool(name="dpool", bufs=4, space="DRAM") as dpool,
            tc.tile_pool(name="ps_a", bufs=2, space="PSUM") as ps_a,
            tc.tile_pool(name="ps_s", bufs=2, space="PSUM") as ps_s,
            tc.tile_pool(name="ps_o", bufs=1, space="PSUM") as ps_o,
            tc.tile_pool(name="ps_d", bufs=1, space="PSUM") as ps_d,
        ):
            # ---- constants / weights resident in SBUF ----
            mask_sb = cpool.tile([128, 896], f16, name="mask_sb")
            nc.sync.dma_start(out=mask_sb, in_=msk)
            ones_sb = cpool.tile([128, 1], f16, name="ones_sb")
            nc.vector.memset(ones_sb, 1.0)
            # Selection matrices for the denominator partition-broadcast:
            # sel[:, 0:128] (even pairs) routes PSUM row 0 -> out partitions
            # 0:64 and row 32 -> 64:128; sel[:, 128:256] (odd pairs) routes
            # rows 64/96 likewise.  out = sel.T @ recip_rows.
            sel_sb = cpool.tile([128, 256], f16, name="sel_sb")
            nc.vector.memset(sel_sb, 0.0)
            nc.vector.memset(sel_sb[0:1, 0:64], 1.0)
            nc.vector.memset(sel_sb[32:33, 64:128], 1.0)
            nc.vector.memset(sel_sb[64:65, 128:192], 1.0)
            nc.vector.memset(sel_sb[96:97, 192:256], 1.0)

            # DMA order: the head-critical subset first -- q/k weight
            # chunks (2KB lines) interleaved with the stage-0 x^T slices,
            # consumed incrementally by the first q/k chains -- then wv,
            # the remaining x^T, and w_out stream in behind.  Chunk DMAs
            # are batched in pairs/quads via 3D APs: each DMA instruction
            # costs ~600ns of SP-engine issue time, so 44 small transfers
            # would burn ~10us at the head before any byte moves.
            wqkv_big = wpool.tile([128, 8 * 1536], f16, name="wqkv", tag="w")
            xt_big = xpool.tile([128, 8 * 2048], f16, name="xtb", tag="xt")
            wqkv_sb = [
                wqkv_big[:, cc * 1536 : (cc + 1) * 1536] for cc in range(CCH)
            ]
            xt_sb = [
                xt_big[:, cc * 2048 : (cc + 1) * 2048] for cc in range(CCH)
            ]
            for cc in range(CCH):
                nc.sync.dma_start(
                    out=wqkv_sb[cc][:, 0:1024],
                    in_=wqk[cc * 128 : (cc + 1) * 128, :],
                )
                nc.sync.dma_start(
                    out=xt_sb[cc][:, 0:512],
                    in_=xt[cc * 128 : (cc + 1) * 128, 0:512],
                )
            for cc in range(CCH):
                nc.sync.dma_start(
                    out=wqkv_sb[cc][:, 1024:1536],
                    in_=wv[cc * 128 : (cc + 1) * 128, :],
                )
                nc.sync.dma_start(
                    out=xt_sb[cc][:, 512:1024],
                    in_=xt[cc * 128 : (cc + 1) * 128, 512:1024],
                )
            for cc in range(CCH):
                nc.sync.dma_start(
                    out=xt_sb[cc][:, 1024:2048],
                    in_=xt[cc * 128 : (cc + 1) * 128, 1024:2048],
                )
            wo_big = wopool.tile([128, 4 * C], f16, name="wo", tag="wo")
            wo_sb = [wo_big[:, fc * C : (fc + 1) * C] for fc in range(4)]
            for fc in range(4):
                nc.sync.dma_start(
                    out=wo_sb[fc], in_=wo[fc * 128 : (fc + 1) * 128, :]
                )

            # ---- persistent activations ----
            qT = [
                qkpool.tile([128, T], f16, name=f"qT_{p}", tag="qk")
                for p in range(PAIRS)
            ]
            kT = [
                qkpool.tile([128, T], f16, name=f"kT_{p}", tag="qk")
                for p in range(PAIRS)
            ]
            v_sb = [
                vpool.tile([128, 512], f16, name=f"v_{j}", tag="v")
                for j in range(NJT)
            ]
            att = [
                apool.tile([128, T], f16, name=f"att_{p}", tag="att")
                for p in range(PAIRS)
            ]

            def keepalive():
                """Standalone 32-column LDWEIGHTS: marks the PE busy for
                the HAM activity monitor (so scalar-paced stretches don't
                re-throttle the clock to half rate) without touching PSUM
                or creating dependencies.  The next matmul self-loads its
                weights, so clobbering the array is harmless."""
                nc.tensor.ldweights(mask_sb[:, 0:32])

            def qk_chain(t4, base, dst, n):
                """One q- or k-projection chain: pair n, token chunk t4."""
                tsl4 = slice(t4 * 512, (t4 + 1) * 512)
                ps = ps_a.tile([128, 512], f32, name="ps_qk", tag="psA")
                for cc in range(CCH):
                    nc.tensor.matmul(
                        ps,
                        lhsT=wqkv_sb[cc][
                            :, base + n * 128 : base + (n + 1) * 128
                        ],
                        rhs=xt_sb[cc][:, tsl4],
                        start=(cc == 0),
                        stop=(cc == CCH - 1),
                    )
                nc.vector.tensor_copy(dst[n][:, tsl4], ps)

            def v_chain(t4, tt):
                """One v-projection chain: token tile t4*4+tt."""
                tsl4 = slice(t4 * 512, (t4 + 1) * 512)
                ps = ps_a.tile([128, 512], f32, name="ps_v", tag="psA")
                for cc in range(CCH):
                    nc.tensor.matmul(
                        ps,
                        lhsT=xt_sb[cc][
                            :, t4 * 512 + tt * 128 : t4 * 512 + (tt + 1) * 128
                        ],
                        rhs=wqkv_sb[cc][:, 1024:1536],
                        start=(cc == 0),
                        stop=(cc == CCH - 1),
                    )
                nc.vector.tensor_copy(v_sb[t4 * 4 + tt], ps)

            def qkv_chains(t4):
                """QKV projection chains for token chunk t4 (512 tokens);
                yields once per PSUM chain so chains can interleave with
                phase_b units."""
                for base, dst in ((0, qT), (512, kT)):
                    for n in range(PAIRS):
                        qk_chain(t4, base, dst, n)
                        yield
                for tt in range(4):
                    v_chain(t4, tt)
                    yield

            def proj_chains(s):
                """Output projection chains for token tiles 4s..4s+4;
                yields once per (token tile, out half) chain."""
                for tt in range(4 * s, 4 * s + 4):
                    tsl = slice(tt * 128, (tt + 1) * 128)
                    for n in range(2):
                        keepalive()
                        ps = ps_a.tile([128, 512], f32, name="ps_c", tag="psA")
                        for fc in range(4):
                            nc.tensor.matmul(
                                ps,
                                lhsT=att[fc][:, tsl],
                                rhs=wo_sb[fc][:, n * 512 : (n + 1) * 512],
                                start=(fc == 0),
                                stop=(fc == 3),
                            )
                        ost = opool.tile([128, 512], f32, name="ost", tag="ost")
                        nc.vector.tensor_copy(ost, ps)
                        nc.sync.dma_start(
                            out=out[tsl, n * 512 : (n + 1) * 512], in_=ost
                        )
                        yield

            def phase_b_group(ic, g, dbanks, deferred_muls=None, norm_out=None):
                """Attention for query chunk ic (512 queries), pair group g
                (pairs 2g, 2g+1).  The two pairs' QK/exp streams interleave
                per key tile; pair A's PV accumulates inline while pair B's
                pT tiles are kept in SBUF and its PV runs as one dense
                burst after the loop (one dependency stall instead of one
                per key tile).  Yields once per key-tile step."""
                isl = slice(ic * 512, (ic + 1) * 512)
                njt = 4 * ic + 4
                prA, prB = 2 * g, 2 * g + 1

                dbanks[g] = ps_d.tile(
                    [128, 512], f32, name=f"ps_den{g}", tag="psd"
                )
                dbank = dbanks[g]
                if ic == 0 and g == 0:
                    # Virgin PSUM may hold NaN/Inf bit patterns; the
                    # broadcast matmul contracts over rows 0:97 of this
                    # bank (x * 0 = NaN for non-finite x), so establish
                    # finite contents once.  Every later reuse is
                    # overwritten by a broadcast matmul.
                    nc.vector.memset(dbank, 1.0)

                def geom(jt):
                    dpos = jt - 4 * ic
                    ioff = 128 * dpos if dpos > 0 else 0
                    return dpos, ioff

                def qk_exp(pr, jt, tag):
                    """Scores + exp + mask + pacc update for one pair/key
                    tile; returns the pT tile for the PV."""
                    dpos, ioff = geom(jt)
                    jsl = slice(jt * 128, (jt + 1) * 128)
                    islq = slice(ic * 512 + ioff, (ic + 1) * 512)
                    sb = ps_s.tile([128, 1024], f32, name="sb", tag="pss")
                    nc.tensor.matmul(
                        sb[:, ioff:512],
                        lhsT=kT[pr][0:64, jsl],
                        rhs=qT[pr][0:64, islq],
                        start=True,
                        stop=True,
                        tile_position=(0, 0),
                    )
                    nc.tensor.matmul(
                        sb[:, 512 + ioff : 1024],
                        lhsT=kT[pr][64:128, jsl],
                        rhs=qT[pr][64:128, islq],
                        start=True,
                        stop=True,
                        tile_position=(64, 0),
                    )
                    pTb = ppool.tile(
                        [128, 1024], f16, name="pTb", tag=tag,
                        bufs=(18 if tag == "pTb" else 5),
                    )
                    # One exp covers both heads; on diagonal tiles a 3D AP
                    # skips the dead zone between the halves.
                    if dpos > 0:
                        s3 = sb[:, :].rearrange("p (h w) -> p h w", h=2)[
                            :, :, ioff:512
                        ]
                        p3e = pTb[:, :].rearrange("p (h w) -> p h w", h=2)[
                            :, :, ioff:512
                        ]
                        nc.scalar.activation(p3e, s3, EXP, scale=0.125)
                    else:
                        nc.scalar.activation(pTb, sb, EXP, scale=0.125)
                    return pTb

                def mask_pacc(pr, jt, pTb):
                    """Causal mask + softmax-partial accumulation for one
                    pair/key tile (DVE work gated on the exp)."""
                    dpos, ioff = geom(jt)
                    if dpos >= 0:
                        # Mask both heads' 128-wide triangle bands in one
                        # 3D-AP multiply (columns beyond the band are fully
                        # valid; columns below ioff are never read).
                        p3 = pTb[:, :].rearrange("p (h w) -> p h w", h=2)[
                            :, :, ioff : ioff + 128
                        ]
                        m3 = (
                            mask_sb[:, 384:512]
                            .unsqueeze(1)
                            .broadcast_to([128, 2, 128])
                        )
                        nc.vector.tensor_mul(p3, p3, m3)
                    pacc = paccs[pr]
                    if jt == 0:
                        nc.vector.tensor_copy(pacc, pTb)
                    elif dpos > 0:
                        a3 = pacc[:, :].rearrange("p (h w) -> p h w", h=2)[
                            :, :, ioff:512
                        ]
                        q3 = pTb[:, :].rearrange("p (h w) -> p h w", h=2)[
                            :, :, ioff:512
                        ]
                        nc.vector.tensor_add(a3, a3, q3)
                    else:
                        nc.vector.tensor_add(pacc, pacc, pTb)

                def pv(pr, jt, ps_out, pTb, first, last):
                    _, ioff = geom(jt)
                    vt = v_sb[jt]
                    nc.tensor.matmul(
                        ps_out[0:64, ioff:512],
                        lhsT=vt[:, pr * 128 : pr * 128 + 64],
                        rhs=pTb[:, ioff:512],
                        start=first,
                        stop=False,
                        tile_position=(0, 0),
                        skip_group_check=True,
                    )
                    nc.tensor.matmul(
                        ps_out[64:128, ioff:512],
                        lhsT=vt[:, pr * 128 + 64 : pr * 128 + 128],
                        rhs=pTb[:, 512 + ioff : 1024],
                        start=first,
                        stop=last,
                        tile_position=(0, 64),
                        skip_group_check=True,
                    )

                def den(pr):
                    dp0 = 64 * (pr % 2)
                    dp1 = dp0 + 32
                    pacc = paccs[pr]
                    nc.tensor.matmul(
                        dbank[dp0 : dp0 + 1, :],
                        lhsT=ones_sb,
                        rhs=pacc[:, 0:512],
                        start=True,
                        stop=True,
                        tile_position=(0, dp0),
                        skip_group_check=True,
                    )
                    nc.tensor.matmul(
                        dbank[dp1 : dp1 + 1, :],
                        lhsT=ones_sb,
                        rhs=pacc[:, 512:1024],
                        start=True,
                        stop=True,
                        tile_position=(0, dp1),
                        skip_group_check=True,
                    )

                def norm_group(ps_out_B=None):
                    """1/denominators for both pairs -> broadcast +
                    in-place multiply into att.  If ps_out_B is given,
                    pair B's evacuation is fused with its normalization
                    (att_B = ps_out_B * rdb in one pass)."""
                    rec = rpool.tile([128, 512], f32, name="rec", tag="rec", bufs=2)
                    nc.vector.reciprocal_approx_fast(
                        out=rec[0:97, :], in_=dbank[0:97, :]
                    )
                    rech = rpool.tile([128, 512], f16, name="rech", tag="rech", bufs=2)
                    nc.vector.tensor_copy(rech[0:97, :], rec[0:97, :])
                    # Partition-broadcast via PE: sel.T @ rech replicates
                    # each pair's two reciprocal rows across 64 partitions
                    # each, written back over the (already-consumed) den
                    # bank.  The in-place multiply reads PSUM directly.
                    for lp, pr in ((0, prA), (1, prB)):
                        nc.tensor.matmul(
                            dbank,
                            lhsT=sel_sb[0:97, 128 * lp : 128 * lp + 128],
                            rhs=rech[0:97, :],
                            start=True,
                            stop=True,
                            skip_group_check=True,
                        )
                        asl = att[pr][:, isl]
                        nc.vector.tensor_mul(asl, asl, dbank)

                def norm_group_deferred():
                    """Like norm_group, but the broadcast reciprocals land
                    in SBUF (DRAM bounce) and the in-place att multiplies
                    are returned as closures.  The caller weaves those into
                    a LATER stage's instruction streams: the in-order
                    engine queues then gate the dependent out-projection
                    chains into that stage, giving the PE filler work right
                    where ScalarE-paced attention would otherwise leave it
                    idle (and HAM-throttled)."""
                    rec = rpool.tile([128, 512], f32, name="rec", tag="rec", bufs=2)
                    nc.vector.reciprocal_approx_fast(
                        out=rec[0:97, :], in_=dbank[0:97, :]
                    )
                    rech = rpool.tile([128, 512], f16, name="rech", tag="rech", bufs=2)
                    nc.vector.tensor_copy(rech[0:97, :], rec[0:97, :])
                    dsc = dpool.tile([4, 512], f16, name="dsc", tag="dsc")
                    for r in range(4):
                        nc.sync.dma_start(
                            out=dsc[r : r + 1, :],
                            in_=rech[32 * r : 32 * r + 1, :],
                        )
                    muls = []
                    for lp, pr in ((0, prA), (1, prB)):
                        rdb = rpool.tile(
                            [128, 512], f16, name="rdb", tag="rdb", bufs=8
                        )
                        nc.sync.dma_start(
                            out=rdb[0:64, :],
                            in_=dsc[2 * lp : 2 * lp + 1, :].broadcast_to(
                                [64, 512]
                            ),
                        )
                        nc.sync.dma_start(
                            out=rdb[64:128, :],
                            in_=dsc[2 * lp + 1 : 2 * lp + 2, :].broadcast_to(
                                [64, 512]
                            ),
                        )

                        def mul(pr=pr, rdb=rdb):
                            asl = att[pr][:, isl]
                            nc.vector.tensor_mul(asl, asl, rdb)

                        muls.append(mul)
                    return muls

                paccs = {
                    pr: qpool.tile([128, 1024], f16, name="pacc", tag="pacc")
                    for pr in (prA, prB)
                }
                if ic == 0:
                    # Head special case: run pair A's 4 QK/exp steps before
                    # pair B's (first exps then only need pair A's q/k
                    # chains, which need only the first 2MB of DMA), with
                    # both PVs deferred to bursts.
                    pT_A = []
                    for jt in range(njt):
                        pT_A.append(qk_exp(prA, jt, "pT"))
                        mask_pacc(prA, jt, pT_A[-1])
                        yield
                    pT_B = []
                    for jt in range(njt):
                        pT_B.append(qk_exp(prB, jt, "pTb"))
                        mask_pacc(prB, jt, pT_B[-1])
                        yield
                    ps_out_A = ps_o.tile([128, 512], f32, name="ps_out", tag="pso")
                    for jt, pTa in enumerate(pT_A):
                        pv(prA, jt, ps_out_A, pTa, jt == 0, jt == njt - 1)
                else:
                    ps_out_A = ps_o.tile([128, 512], f32, name="ps_out", tag="pso")
                    pT_B = []
                    lag_a = []
                    for jt in range(njt):
                        pTa = qk_exp(prA, jt, "pT")
                        mask_pacc(prA, jt, pTa)
                        # PV_A lags two key tiles and pair B's mask/pacc
                        # one key tile behind their exps, so neither
                        # in-order engine queue stalls mid-step on an exp
                        # semaphore.
                        lag_a.append((jt, pTa))
                        if len(lag_a) > 2:
                            j0, p0 = lag_a.pop(0)
                            pv(prA, j0, ps_out_A, p0, j0 == 0, False)
                        if jt > 0:
                            mask_pacc(prB, jt - 1, pT_B[-1])
                        pTb = qk_exp(prB, jt, "pTb")
                        pT_B.append(pTb)
                        if ic >= 2:
                            keepalive()
                        yield
                    mask_pacc(prB, njt - 1, pT_B[-1])
                    for j0, p0 in lag_a:
                        pv(prA, j0, ps_out_A, p0, j0 == 0, j0 == njt - 1)
                nc.vector.tensor_copy(att[prA][:, isl], ps_out_A)
                # Dens between evac_A and the burst: their pacc deps have
                # retired, and they fill the PE's wait on evac_A (the PSUM
                # ring gate for the burst's output bank).
                den(prA)
                den(prB)
                ps_out_B = ps_o.tile([128, 512], f32, name="ps_out", tag="pso")
                for jt, pTb in enumerate(pT_B):
                    pv(prB, jt, ps_out_B, pTb, jt == 0, jt == njt - 1)
                nc.vector.tensor_copy(att[prB][:, isl], ps_out_B)
                if norm_out is not None:
                    norm_out.append(norm_group)
                elif deferred_muls is None:
                    norm_group()
                else:
                    deferred_muls.extend(norm_group_deferred())

            def phase_b_units(ic, deferred_muls=None):
                """All pair groups of phase_b(ic), in group order.  Group
                0's normalization is deferred past group 1's first step so
                its sel-matmuls (gated on the DVE reciprocal chain) don't
                block group 1's ready QK matmuls in the in-order PE
                queue."""
                dbanks = [None, None]
                pend = []
                for _ in phase_b_group(ic, 0, dbanks, deferred_muls, pend):
                    yield
                first = True
                for _ in phase_b_group(ic, 1, dbanks, deferred_muls):
                    yield
                    if first and pend:
                        pend.pop(0)()
                        first = False

            def run_stage(s, filler_steps, deferred_muls=None):
                """Emit phase_b(s) steps with filler chains spread evenly
                between them (emission order = scheduler priority)."""
                U = 8 * (s + 1)
                F = len(filler_steps)
                k = 0
                for i, _ in enumerate(
                    phase_b_units(s, deferred_muls), start=1
                ):
                    while k < (F * i) // U:
                        filler_steps[k]()
                        k += 1
                while k < F:
                    filler_steps[k]()
                    k += 1

            def steps_of(gen, n):
                g = iter(gen)
                return [lambda g=g: next(g, None) for _ in range(n)]

            def interleaved(a, b):
                res = []
                la, lb = len(a), len(b)
                n = max(la, lb)
                for i in range(n):
                    if i < la:
                        res.append(a[i])
                    if i < lb:
                        res.append(b[i])
                return res

            # ---- global schedule ----
            # Stage 0: emit each pair's attention right after the q/k
            # chains it needs, so the in-order engine queues don't trap the
            # first exps behind DMA-paced projection matmuls.  The first
            # exps need only the q0/k0 chains; everything else (other q/k
            # chains, v chains, QKV(1)) weaves in behind as PE filler.
            db0 = [None, None]
            qk_chain(0, 0, qT, 0)
            qk_chain(0, 512, kT, 0)
            weave0 = [
                lambda: qk_chain(0, 0, qT, 1),
                lambda: qk_chain(0, 512, kT, 1),
                lambda: v_chain(0, 0),
                lambda: v_chain(0, 1),
                lambda: v_chain(0, 2),
                lambda: v_chain(0, 3),
            ]
            qkv1 = steps_of(qkv_chains(1), 12)
            weave1 = [
                lambda: qk_chain(0, 0, qT, 3),
                lambda: qk_chain(0, 512, kT, 3),
            ] + qkv1[0:6]
            for g in range(2):
                if g == 1:
                    qk_chain(0, 0, qT, 2)
                    qk_chain(0, 512, kT, 2)
                weave = weave0 if g == 0 else weave1
                k1 = 0
                for _ in phase_b_group(0, g, db0):
                    if k1 < len(weave):
                        weave[k1]()
                        k1 += 1
                while k1 < len(weave):
                    weave[k1]()
                    k1 += 1
            for st in qkv1[6:12]:
                st()
            run_stage(1, steps_of(qkv_chains(2), 12))
            run_stage(
                2,
                interleaved(
                    steps_of(qkv_chains(3), 12), steps_of(proj_chains(0), 8)
                ),
            )
            run_stage(
                3,
                interleaved(
                    steps_of(proj_chains(1), 8), steps_of(proj_chains(2), 8)
                ),
            )
            for _ in proj_chains(3):
                pass

    nc.compile()
    return nc


def _get_program():
    if "nc" not in _CACHE:
        _CACHE["nc"] = _build_program()
    return _CACHE["nc"]


def _make_mask():
    # msk[jj, z] = 1 if z >= jj + 384 else 0; diagonal-position-p mask
    # tile is msk[:, 384-128p : 384-128p+512].
    jj = np.arange(128)[:, None]
    z = np.arange(896)[None, :]
    return (z >= jj + 384).astype(np.float16)


def _make_in_maps(x, w_qkv, w_out):
    mask = _make_mask()
    in_maps = []
    for core in range(NCORES):
        b, hg = core // 2, core % 2
        cs = slice(hg * 512, (hg + 1) * 512)
        f16 = np.float16
        wqk = np.concatenate(
            [
                w_qkv[:, hg * 512 : hg * 512 + 512],
                w_qkv[:, 1024 + hg * 512 : 1024 + hg * 512 + 512],
            ],
            axis=1,
        )
        in_maps.append(
            {
                "xt": np.ascontiguousarray(x[b].T).astype(f16),
                "wqk": np.ascontiguousarray(wqk).astype(f16),
                "wv": np.ascontiguousarray(
                    w_qkv[:, 2048 + hg * 512 : 2048 + hg * 512 + 512]
                ).astype(f16),
                "wo": np.ascontiguousarray(w_out[cs, :]).astype(f16),
                "msk": mask,
            }
        )
    return in_maps


def _run_device(in_maps, trace=False):
    from concourse.bass_utils import run_bass_kernel_spmd

    nc = _get_program()
    return run_bass_kernel_spmd(
        nc, in_maps, core_ids=list(range(NCORES)), trace=trace
    )


def kernel(x, w_qkv, w_out, b_out):
    x = np.asarray(x, dtype=np.float32)
    w_qkv = np.asarray(w_qkv, dtype=np.float32)
    w_out = np.asarray(w_out, dtype=np.float32)
    b_out = np.asarray(b_out, dtype=np.float32)

    res = _run_device(_make_in_maps(x, w_qkv, w_out)).results
    out = np.empty((B, T, C), dtype=np.float32)
    for b in range(B):
        out[b] = res[2 * b]["out"] + res[2 * b + 1]["out"] + b_out
    return out
